# revision 9
# baseline (speedup 1.0000x reference)
"""AdderNet depthwise 3x3 L1-distance conv for Trainium2, 8-core data parallel, v4.

out[b,c,h,w] = -sum_{i,j in 3x3} |x_pad[b,c,h+i,w+j] - W[c,0,i,j]|

Strategy (per core, 4 images of the batch = 16 (b,c) planes):
- Host converts x to bf16 and repacks input AND output planes into a
  partition-major block layout (idx = 4*q + b for padded row 128b+q, rows
  512/513 appended) so every main DMA moves one contiguous 4-row chunk per
  partition. Output stays f32.
- Planes are processed in CHANNEL-GROUPED PAIRS (two images, same channel):
  one tap instruction covers both planes [128, 2*4, 512]; weights are
  uniform per group and baked into the program as immediates (program
  cache keyed on weight bytes; setup_inputs is deterministic).
- 9 taps per group: the 6 even-j taps as 3 DVE 2-src custom ABS_DIFF2 ops
  (maxx(x0-w0, w0-x0) + maxx(x1-w1, w1-x1): two taps AND their sum in one
  1x pass -> one PE plane per i instead of two), the 3 odd-j taps on ACT
  via activation(Abs, bias=-w) -- except tap (1,1), which is COLUMN-SPLIT:
  ACT does cols 0:384, DVE the rest via tensor_scalar(sub) then
  bitwise-AND 0x7FFF on an int16 view (clears the bf16 sign bit = abs),
  using DVE's residual slack without touching PSUM. ABS_MAX has no TRN2
  TPB encoding, so no fused tensor_scalar absdiff exists on any engine;
  GPSIMD shares its SBUF port with DVE 2-port instructions and cannot
  help.
- TensorE applies the 3 row shifts via shifted-identity stationaries:
  6 planes x 4 blocks = 24 matmuls per image-plane (zigzag i-order across
  groups for LDWEIGHTS locality), PSUM start/stop per bank.
- Evac psum->sbuf f32 with the output negation (scale=-1): all on ACT
  mid-stream (a DVE evac gets scheduled ahead of its deps and stalls),
  DVE-heavy for the final drain; emitted one group late so the tap
  engines never starve.
- Seam rows (2 per 128-row block) recomputed by one merged fixup tile over
  all 16 planes with compact-output shift matrices F; the whole seam
  writeback is 3 DMAs per plane-octet.
- Steady state is jointly DVE+ACT saturated (~14.1us each per 2-plane
  group); measured 8-core HW exec ~152.6us at full clock (the device
  intermittently throttles to 5/6 clock -> ~183us), rel err 3.1e-3.
"""

import numpy as np
import ml_dtypes

B, C, H, W = 32, 4, 512, 512
N_CORES = 8
B_LOC = B // N_CORES          # 4 images per core
N_IMG = B_LOC * C             # 16 (b,c) planes per core
HP, WP = H + 2, W + 2         # 514, 514
NBLK = 4                      # row blocks of 128 per plane
P = 128
GB = 2                        # planes per channel-group (same channel)

PAIRS = ((0, 0, 2), (1, 0, 2), (2, 0, 2))
SINGLES = ((0, 1), (1, 1), (2, 1))
EVAC_DVE_BLOCKS = 1           # psum blocks DVE evacuates per plane (rest ACT)

_PROGRAM_CACHE = {}


def _register_op(name, make_spec):
    from concourse import dve_ops
    from concourse.dve_spec import lower
    from concourse.dve_uop import DveOpSpec

    for o in dve_ops.OPS:
        if o.name == name:
            return o
    spec = make_spec()
    shas = {
        ver: DveOpSpec(name=name, uops=lower(spec, ver=ver)).sha(ver)
        for ver in ("v3", "v4")
    }
    op = dve_ops.DveOp(name, spec, subdim=False, uops_sha=shas)
    dve_ops.OPS.append(op)
    dve_ops.CUSTOM_DVE_SPECS[op.name] = spec
    dve_ops._SUB_OPCODE_FOR_NAME[op.name] = (
        dve_ops._CUSTOM_DVE_ROW_BASE + len(dve_ops.OPS) - 1
    )
    return op


def _absdiff2_op():
    """out = |in0 - s0| + |in1 - s1| (7 ALU stages, 2 tensor srcs)."""
    from concourse.dve_spec import Spec, Src0, Src1, C0, C1, maxx

    def make():
        def _ref(in0, in1, s0, s1, imm2):
            a0 = np.float32(s0)
            a1 = np.float32(s1)
            return (
                np.abs(in0.astype(np.float32) - a0)
                + np.abs(in1.astype(np.float32) - a1)
            ).astype(np.float32)

        return Spec(
            body=maxx(Src0 - C0, C0 - Src0) + maxx(Src1 - C1, C1 - Src1),
            reference=_ref,
        )

    return _register_op("ABS_DIFF2_ANT", make)


def _absdiff1_op():
    """out = |in0 - s0| with s0 a [P,1] AP (for fixup tiles)."""
    from concourse.dve_spec import Spec, Src0, C0, maxx

    def make():
        def _ref(in0, in1, s0, s1, imm2):
            s = np.asarray(s0)
            if s.ndim and in0.ndim > s.ndim:
                s = s.reshape(s.shape[0], *([1] * (in0.ndim - 1)))
            return np.abs(in0.astype(np.float32) - s).astype(np.float32)

        return Spec(body=maxx(Src0 - C0, C0 - Src0), reference=_ref)

    return _register_op("ABS_DIFF_ANT", make)


def _build_program(w9):
    """w9: [C, 9] float32 weights (baked as immediates)."""
    import concourse.mybir as mybir
    import concourse.tile as tile
    from concourse import bacc

    f32 = mybir.dt.float32
    bf16 = mybir.dt.bfloat16
    absdiff2 = _absdiff2_op()
    absdiff1 = _absdiff1_op()
    nc = bacc.Bacc("TRN2", target_bir_lowering=False)

    # xh[st, 4*q+b, :] = xpad[st, 128*b+q, :]; xh[st, 512+k, :] = row 512+k
    xh = nc.declare_dram_parameter("xh", [N_IMG, 4 * P + 2, WP], bf16, isOutput=False)
    smat = nc.declare_dram_parameter("smat", [3, P, P], bf16, isOutput=False)
    fmat = nc.declare_dram_parameter("fmat", [3, P, 64], bf16, isOutput=False)
    # bias: cols c*9+t = -w (main ACT taps), then 2 fixup tiles x 18
    bias = nc.declare_dram_parameter("bias", [P, C * 9 + 18], f32, isOutput=False)
    # oh: same idx scheme as xh (padded rows), host unpacks
    oh = nc.declare_dram_parameter("oh", [N_IMG, 4 * P + 2, W], f32, isOutput=True)

    Abs = mybir.ActivationFunctionType.Abs
    Copy = mybir.ActivationFunctionType.Copy

    groups = [
        (c, (2 * bp * C + c, (2 * bp + 1) * C + c))
        for c in range(C)
        for bp in range(B_LOC // 2)
    ]

    with tile.TileContext(nc) as tc:
        with (
            tc.tile_pool(name="const", bufs=1) as cpool,
            tc.tile_pool(name="xp", bufs=3) as xpool,
            tc.tile_pool(name="dp", bufs=13) as dpool,
            tc.tile_pool(name="op", bufs=6) as opool,
            tc.tile_pool(name="ps", bufs=2, space="PSUM") as ppool,
        ):
            def _load_x2(sts):
                x2 = xpool.tile([P, GB * NBLK, WP], bf16, tag="x")
                for k, st in enumerate(sts):
                    nc.sync.dma_start(
                        out=x2[:, k * NBLK : (k + 1) * NBLK],
                        in_=xh[st, 0 : 4 * P, :].rearrange("(q b) w -> q b w", q=P),
                    )
                return x2

            # prefetch the first three groups' inputs before the constants
            pre_x = [_load_x2(sts0) for (_, sts0) in groups[:3]]

            s_t = cpool.tile([P, 3, P], bf16, tag="s")
            nc.sync.dma_start(out=s_t, in_=smat[:].rearrange("s k p -> k s p"))
            f_t = cpool.tile([P, 3, 64], bf16, tag="f")
            nc.sync.dma_start(out=f_t, in_=fmat[:].rearrange("s k p -> k s p"))
            b_all = cpool.tile([P, C * 9 + 18], f32, tag="ball")
            nc.sync.dma_start(out=b_all, in_=bias[:])

            warm = cpool.tile([P, 2], f32, tag="warm")
            nc.vector.memset(warm, 0.0)
            nc.scalar.activation(
                out=warm[:, 0:1], in_=warm[:, 1:2], func=Abs, bias=0.0, scale=1.0
            )
            nc.scalar.activation(
                out=warm[:, 1:2], in_=warm[:, 0:1], func=Copy, scale=-1.0
            )

            def _evac(ps, st, eb):
                o_t = opool.tile([P, NBLK, W], f32, tag="o")
                if eb:
                    nc.vector.tensor_scalar(
                        out=o_t[:, :eb],
                        in0=ps[:, :eb],
                        scalar1=-1.0,
                        scalar2=None,
                        op0=mybir.AluOpType.mult,
                    )
                nc.scalar.activation(
                    out=o_t[:, eb:], in_=ps[:, eb:], func=Copy, scale=-1.0
                )
                nc.sync.dma_start(
                    out=oh[st, 0 : 4 * P, :].rearrange("(q b) w -> q b w", q=P)[1:127],
                    in_=o_t[1:127],
                )

            KSP = 384  # columns of tap (1,1) done by ACT; DVE does the rest

            def _taps(c, x2):
                planes = []
                for (i, j) in SINGLES:
                    d = dpool.tile([P, GB * NBLK, W], bf16, tag="d")
                    if (i, j) == (1, 1):
                        # column-split: ACT does [0:KSP], DVE the tail via
                        # sub then bitwise-AND 0x7FFF (clears the bf16 sign
                        # bit = abs) on an int16 view -- uses DVE's slack
                        # without touching PSUM.
                        nc.scalar.activation(
                            out=d[:, :, 0:KSP],
                            in_=x2[:, :, j : j + KSP],
                            func=Abs,
                            bias=b_all[:, c * 9 + 4 : c * 9 + 5],
                            scale=1.0,
                        )
                        tmp = dpool.tile(
                            [P, GB * NBLK, W - KSP], bf16, tag="tmp", name="tmp"
                        )
                        nc.vector.tensor_scalar(
                            out=tmp,
                            in0=x2[:, :, j + KSP : j + W],
                            scalar1=float(w9[c, 4]),
                            scalar2=None,
                            op0=mybir.AluOpType.subtract,
                        )
                        nc.vector.tensor_scalar(
                            out=d[:, :, KSP:W].bitcast(mybir.dt.int16),
                            in0=tmp.bitcast(mybir.dt.int16),
                            scalar1=0x7FFF,
                            scalar2=None,
                            op0=mybir.AluOpType.bitwise_and,
                        )
                    else:
                        nc.scalar.activation(
                            out=d,
                            in_=x2[:, :, j : j + W],
                            func=Abs,
                            bias=b_all[:, c * 9 + 3 * i + j : c * 9 + 3 * i + j + 1],
                            scale=1.0,
                        )
                    planes.append((i, d))
                for (i, jA, jB) in PAIRS:
                    d = dpool.tile([P, GB * NBLK, W], bf16, tag="d")
                    nc.vector._custom_dve(
                        absdiff2,
                        out=d,
                        in0=x2[:, :, jA : jA + W],
                        in1=x2[:, :, jB : jB + W],
                        s0=float(w9[c, 3 * i + jA]),
                        s1=float(w9[c, 3 * i + jB]),
                    )
                    planes.append((i, d))
                planes.sort(key=lambda t: t[0])
                return planes

            def _fixup():
                """Seam rows for all 16 planes as one [P, 2, .] batch
                (free dim fi = plane octet). xf partitions
                p = 32*band + 8*rr + g hold padded row 126+128*band+rr of
                plane 8*fi+g; F maps them to compact output partitions.
                Channel of partition g is g%C for BOTH octets, so one
                18-col bias set serves both."""
                xf = xpool.tile([P, 2, WP], bf16, tag="x")
                for fi in range(2):
                    g0 = fi * 8
                    xq = xh[g0 : g0 + 8, 0 : 4 * P, :].rearrange(
                        "g (q b) w -> g q b w", b=4
                    )
                    for band in range(4):
                        nc.sync.dma_start(
                            out=xf[32 * band : 32 * band + 16, fi],
                            in_=xq[:, 126:128, band, :].rearrange("g q w -> q g w"),
                        )
                        if band < 3:
                            nc.sync.dma_start(
                                out=xf[32 * band + 16 : 32 * band + 32, fi],
                                in_=xq[:, 0:2, band + 1, :].rearrange(
                                    "g q w -> q g w"
                                ),
                            )
                        else:
                            nc.sync.dma_start(
                                out=xf[32 * band + 16 : 32 * band + 32, fi],
                                in_=xh[g0 : g0 + 8, 512:514, :].rearrange(
                                    "g q w -> q g w"
                                ),
                            )
                bofs = C * 9
                df_tiles = []
                for t in range(9):
                    i, j = divmod(t, 3)
                    d = dpool.tile([P, 2, W], bf16, tag="d")
                    src = xf[:, :, j : j + W]
                    if t in (0, 2, 4, 8):
                        nc.scalar.activation(
                            out=d,
                            in_=src,
                            func=Abs,
                            bias=b_all[:, bofs + 9 + t : bofs + 10 + t],
                            scale=1.0,
                        )
                    else:
                        nc.vector._custom_dve(
                            absdiff1,
                            out=d,
                            in0=src,
                            s0=b_all[:, bofs + t : bofs + t + 1],
                        )
                    df_tiles.append(d)

                pf = ppool.tile([P, 2, W], mybir.dt.float32, tag="ps")
                for i in range(3):
                    t0 = 3 * i
                    for j in range(3):
                        for fi in range(2):
                            nc.tensor.matmul(
                                pf[0:64, fi],
                                lhsT=f_t[:, i, :],
                                rhs=df_tiles[t0 + j][:, fi],
                                start=(t0 + j == 0),
                                stop=(t0 + j == 8),
                            )

                of = opool.tile([P, 2, W], f32, tag="o")
                nc.vector.tensor_scalar(
                    out=of[0:64],
                    in0=pf[0:64],
                    scalar1=-1.0,
                    scalar2=None,
                    op0=mybir.AluOpType.mult,
                )
                for fi in range(2):
                    g0 = fi * 8
                    # r=0 -> padded rows 127+128*band = idx 508..511 (k = 4g+band)
                    nc.sync.dma_start(
                        out=oh[g0 : g0 + 8, 508:512, :], in_=of[0:32, fi]
                    )
                    # r=1, bands 0-2 -> padded 128*(band+1) = idx 1..3
                    nc.sync.dma_start(out=oh[g0 : g0 + 8, 1:4, :], in_=of[32:56, fi])
                    # r=1, band 3 -> padded row 512 = idx 512 (k = 56+g)
                    nc.sync.dma_start(
                        out=oh[g0 : g0 + 8, 512:513, :], in_=of[56:64, fi]
                    )

            pending = []
            for gi, (c, sts) in enumerate(groups):
                x2 = pre_x[gi] if gi < len(pre_x) else _load_x2(sts)
                planes = _taps(c, x2)

                for ps, st in pending:
                    _evac(ps, st, 0)
                pending = []

                seq = planes if gi % 2 == 0 else planes[::-1]
                n_pl = len(seq)
                for k, st in enumerate(sts):
                    ps = ppool.tile([P, NBLK, W], mybir.dt.float32, tag="ps")
                    for pi, (i, d) in enumerate(seq):
                        for blk in range(NBLK):
                            nc.tensor.matmul(
                                ps[:, blk, :],
                                lhsT=s_t[:, i, :],
                                rhs=d[:, k * NBLK + blk, :],
                                start=(pi == 0),
                                stop=(pi == n_pl - 1),
                            )
                    pending.append((ps, st))

            for ps, st in pending:
                _evac(ps, st, 2)
            pending = []
            _fixup()
    nc.finalize()
    return nc


def _get_program(w9):
    key = w9.tobytes()
    if key not in _PROGRAM_CACHE:
        _PROGRAM_CACHE[key] = _build_program(w9)
    return _PROGRAM_CACHE[key]


def _host_consts(weight):
    w9 = np.asarray(weight, np.float32).reshape(C, 9)

    S = np.zeros((3, P, P), np.float32)
    for i in range(3):
        for p in range(P):
            k = p + i - 1
            if 0 <= k < P:
                S[i, k, p] = 1.0
    S = S.astype(ml_dtypes.bfloat16)

    # F[i, p, k]: tap partition p = 32*band + 8*(r+i) + g contributes to
    # compact out k: r=0 -> 4g+band ; r=1 -> 32+3g+band (band<3) ; 56+g (band 3)
    F = np.zeros((3, P, 64), np.float32)
    for i in range(3):
        for band in range(4):
            for g in range(8):
                p0 = 32 * band + 8 * i + g          # r = 0
                F[i, p0, 4 * g + band] = 1.0
                p1 = 32 * band + 8 * (1 + i) + g    # r = 1
                if band < 3:
                    F[i, p1, 32 + 3 * g + band] = 1.0
                else:
                    F[i, p1, 56 + g] = 1.0
    F = F.astype(ml_dtypes.bfloat16)

    bias = np.zeros((P, C * 9 + 18), np.float32)
    for c in range(C):
        bias[:, c * 9 : c * 9 + 9] = -w9[c][None, :]
    # fixup partitions p = 32*band + 8*rr + g : channel = g % C (both octets)
    o = C * 9
    for band in range(4):
        for rr in range(4):
            for g in range(8):
                c = g % C
                p = 32 * band + 8 * rr + g
                bias[p, o : o + 9] = w9[c]
                bias[p, o + 9 : o + 18] = -w9[c]
    return S, F, bias, w9


def _pack_xh(xpad_shard):
    """[N_IMG, 514, WP] bf16 -> partition-major block layout [N_IMG, 514, WP]."""
    n = xpad_shard.shape[0]
    out = np.empty_like(xpad_shard)
    main = xpad_shard[:, 0:512].reshape(n, 4, P, WP)
    out[:, 0 : 4 * P] = main.transpose(0, 2, 1, 3).reshape(n, 4 * P, WP)
    out[:, 4 * P :] = xpad_shard[:, 512:514]
    return np.ascontiguousarray(out)


def kernel(input, weight):
    from concourse.bass_utils import run_bass_kernel_spmd

    x = np.asarray(input, np.float32)
    S, F, bias, w9 = _host_consts(weight)

    xpad = np.pad(x, ((0, 0), (0, 0), (1, 1), (1, 1))).astype(ml_dtypes.bfloat16)
    in_maps = []
    for core in range(N_CORES):
        shard = xpad[core * B_LOC : (core + 1) * B_LOC].reshape(N_IMG, HP, WP)
        in_maps.append(
            {"xh": _pack_xh(shard), "smat": S, "fmat": F, "bias": bias}
        )

    nc = _get_program(w9)
    res = run_bass_kernel_spmd(nc, in_maps, core_ids=list(range(N_CORES)))

    out = np.empty((B, C, H, W), np.float32)
    for core in range(N_CORES):
        o = res.results[core]["oh"].reshape(N_IMG, 4 * P + 2, W)
        padded = np.empty((N_IMG, HP, W), np.float32)
        padded[:, 0:512] = (
            o[:, 0 : 4 * P].reshape(N_IMG, P, 4, W).transpose(0, 2, 1, 3)
            .reshape(N_IMG, 512, W)
        )
        padded[:, 512:514] = o[:, 4 * P :]
        out[core * B_LOC : (core + 1) * B_LOC] = padded.reshape(
            B_LOC, C, HP, W
        )[:, :, 1 : H + 1, :]
    return out
